# revision 5
# baseline (speedup 1.0000x reference)
"""ChildSum TreeLSTM (complete binary tree, 1023 nodes) on 8 trn2 NeuronCores.

Strategy: the 1023-node complete binary tree splits into 8 independent
127-node subtrees rooted at the 8 nodes of the level with 8 nodes; each
NeuronCore runs one subtree (data-parallel, zero cross-core traffic).  The
top 7 nodes are finished on the host in fp32.  On device, activations live
in transposed layout [H=300 -> 3 partition-chunks of 128, nodes] so the
level recurrence needs no transposes and child pair-sums are stride-2
free-dim adds.  Gate pre-activations (x-projections + biases) are computed
once into a persistent PSUM tensor by upfront matmuls (bias folded in via a
ones-row on the input), and the per-level hidden-state matmuls accumulate
into the same PSUM region, so each level needs a single fused sigmoid over
all three gate groups.  Matmul operands are bf16; cell state and elementwise
math are fp32.
"""

import os
import sys

for _p in ("/opt/trn_rl_repo",):
    if _p not in sys.path and os.path.isdir(_p):
        sys.path.append(_p)

import numpy as np
import ml_dtypes

V = 100000
E = 300
H = 300
LBL = 5
N = 1023
NCORES = 8
LPC = 64  # leaves per core
LOCAL_N = 127  # nodes per core subtree
SIZES = [64, 32, 16, 8, 4, 2, 1]  # per-core level sizes
BASES = [0, 64, 96, 112, 120, 124, 126]  # local level start offsets

KD = (128, 128, 44)  # K-chunk sizes for H=300
KD_AUG = (128, 128, 65)  # K-chunks for augmented inputs (321 rows: 300 real + 20 zero + ones row at 320)
MD = (128, 128, 44)  # M-chunk sizes for H=300 outputs
AUG = 321  # augmented input rows: 300 real + 20 zero pad + 1 ones row

_compiled = {}
LAST_RESULTS = None  # BassKernelResults of the most recent device run


def _build_bass():
    import concourse.bacc as bacc
    import concourse.mybir as mybir
    import concourse.tile as tile

    f32 = mybir.dt.float32
    bf16 = mybir.dt.bfloat16
    AF = mybir.ActivationFunctionType

    nc = bacc.Bacc()

    # DRAM I/O (per-core shapes; SPMD with per-core in_maps)
    xt_d = nc.dram_tensor("xt", [AUG, LOCAL_N], bf16, kind="ExternalInput")
    wa_d = {
        g: nc.dram_tensor(f"wa_{g}", [AUG, H], bf16, kind="ExternalInput")
        for g in ("i", "u", "f")
    }
    wh_d = {
        g: nc.dram_tensor(f"wh_{g}", [H, H], bf16, kind="ExternalInput")
        for g in ("i", "u", "f")
    }
    wo_d = nc.dram_tensor("wo", [AUG, LBL], bf16, kind="ExternalInput")
    logp_d = nc.dram_tensor("logp", [LOCAL_N, LBL], f32, kind="ExternalOutput")
    rooth_d = nc.dram_tensor("rooth", [128, 3], f32, kind="ExternalOutput")
    rootc_d = nc.dram_tensor("rootc", [128, 3], f32, kind="ExternalOutput")

    with tile.TileContext(nc) as tc:
        with (
            tc.tile_pool(name="const", bufs=1) as const,
            tc.tile_pool(name="state", bufs=1) as state,
            tc.tile_pool(name="scr", bufs=2) as scr,
            tc.tile_pool(name="pprex", bufs=1, space="PSUM") as pprex,
            tc.tile_pool(name="plg", bufs=1, space="PSUM") as plg,
        ):
            # ---- constants in SBUF ----
            xt = const.tile([128, 3, LOCAL_N], bf16, tag="xt")
            wa = {g: const.tile([128, 3, H], bf16, tag=f"wa{g}", name=f"wa{g}") for g in "iuf"}
            wh = {g: const.tile([128, 3, H], bf16, tag=f"wh{g}", name=f"wh{g}") for g in "iuf"}
            wo = const.tile([128, 3, LBL], bf16, tag="wo")
            for kc in range(3):
                kda = KD_AUG[kc]
                nc.sync.dma_start(
                    out=xt[:kda, kc, :], in_=xt_d[kc * 128 : kc * 128 + kda, :]
                )
                for g in "iuf":
                    nc.sync.dma_start(
                        out=wa[g][:kda, kc, :],
                        in_=wa_d[g][kc * 128 : kc * 128 + kda, :],
                    )
                    kd = KD[kc]
                    nc.sync.dma_start(
                        out=wh[g][:kd, kc, :],
                        in_=wh_d[g][kc * 128 : kc * 128 + kd, :],
                    )
                nc.sync.dma_start(
                    out=wo[:kda, kc, :], in_=wo_d[kc * 128 : kc * 128 + kda, :]
                )

            # ---- persistent state ----
            hb = state.tile([128, 3, LOCAL_N], bf16, tag="hb")  # hidden (bf16)
            cc = state.tile([128, 3, LOCAL_N], f32, tag="cc")  # cell (fp32)
            prexf = state.tile([128, 3, LOCAL_N], f32, tag="prexf")  # fxx+b copy

            # ---- PSUM: gate pre-activations, lanes (gate*3 + mchunk) ----
            prex = pprex.tile([128, 9, 128], f32, tag="prex")
            gp = pprex.tile([128, 3, 2 * SIZES[1]], f32, tag="gp")
            lg = plg.tile([128, LBL], f32, tag="lg")

            # zero the partition rows the 44-row M-chunk matmuls never write
            # (chunk-2 lanes, partitions >= 44); elementwise ops read them and
            # the garbage must at least be initialized/finite.
            for p0 in (32, 64, 96):
                nc.vector.memset(prex[p0 : p0 + 32, 2:9:3, :], 0.0)
                nc.vector.memset(gp[p0 : p0 + 32, 2, :], 0.0)

            # upfront x-projections (+bias via ones row) for all 127 nodes
            for gi, g in enumerate("iuf"):
                for mc in range(3):
                    md = MD[mc]
                    for kc in range(3):
                        kda = KD_AUG[kc]
                        nc.tensor.matmul(
                            prex[:md, gi * 3 + mc, :LOCAL_N],
                            wa[g][:kda, kc, mc * 128 : mc * 128 + md],
                            xt[:kda, kc, :],
                            start=(kc == 0),
                            stop=(kc == 2),
                        )

            # pristine copy of the f-gate x-projection (fxx + b_fx + b_fh):
            # needed per-child later, while the PSUM original accumulates the
            # o-gate recurrence.
            nc.scalar.activation(
                out=prexf[:, :, :], in_=prex[:, 6:9, :LOCAL_N], func=AF.Copy
            )

            # ---- leaves: no children, gates are pure x-projections ----
            iuo_l = scr.tile([128, 9, LPC], f32, tag="iuo")
            nc.scalar.activation(out=iuo_l, in_=prex[:, :, :LPC], func=AF.Sigmoid)
            nc.vector.tensor_mul(cc[:, :, :LPC], iuo_l[:, 0:3, :], iuo_l[:, 3:6, :])
            th_l = scr.tile([128, 3, LPC], f32, tag="th")
            nc.scalar.activation(out=th_l, in_=cc[:, :, :LPC], func=AF.Tanh)
            nc.vector.tensor_mul(hb[:, :, :LPC], iuo_l[:, 6:9, :], th_l)

            # ---- internal levels ----
            for lvl in range(1, 7):
                P = SIZES[lvl]
                C = 2 * P
                cb = BASES[lvl - 1]
                pc = BASES[lvl]

                # child-sum of h (bf16, matmul rhs)
                hsum = scr.tile([128, 3, P], bf16, tag="hsum")
                chv = hb[:, :, cb : cb + C].rearrange(
                    "p c (n two) -> p c n two", two=2
                )
                nc.vector.tensor_add(hsum, chv[:, :, :, 0], chv[:, :, :, 1])

                # i, u, o recurrence matmuls accumulate onto the PSUM
                # pre-activations (o rides the f-gate lanes: fxx + hsum@W_fh)
                for gi, g in enumerate("iuf"):
                    for mc in range(3):
                        md = MD[mc]
                        for kc in range(3):
                            kd = KD[kc]
                            nc.tensor.matmul(
                                prex[:md, gi * 3 + mc, pc : pc + P],
                                wh[g][:kd, kc, mc * 128 : mc * 128 + md],
                                hsum[:kd, kc, :],
                                start=False,
                                stop=(kc == 2),
                                skip_group_check=True,
                            )

                # per-child forget pre-activation g = h_child @ W_fh
                for mc in range(3):
                    md = MD[mc]
                    for kc in range(3):
                        kd = KD[kc]
                        nc.tensor.matmul(
                            gp[:md, mc, :C],
                            wh["f"][:kd, kc, mc * 128 : mc * 128 + md],
                            hb[:kd, kc, cb : cb + C],
                            start=(kc == 0),
                            stop=(kc == 2),
                        )

                # fused sigmoid over i/u/o lanes for this level's columns
                iuo = scr.tile([128, 9, P], f32, tag="iuo")
                nc.scalar.activation(
                    out=iuo, in_=prex[:, :, pc : pc + P], func=AF.Sigmoid
                )

                # f = sigmoid(g + fxx_parent) , parent broadcast to its pair
                gp4 = gp[:, :, :C].rearrange("p c (n two) -> p c n two", two=2)
                nc.vector.tensor_add(
                    gp4, gp4, prexf[:, :, pc : pc + P].to_broadcast((128, 3, P, 2))
                )
                ft = scr.tile([128, 3, C], f32, tag="ft")
                nc.scalar.activation(out=ft, in_=gp[:, :, :C], func=AF.Sigmoid)

                # c = i*u + sum_children f*c_child
                fc = scr.tile([128, 3, C], f32, tag="fc")
                nc.vector.tensor_mul(fc, ft, cc[:, :, cb : cb + C])
                iu = scr.tile([128, 3, P], f32, tag="iu")
                nc.vector.tensor_mul(iu, iuo[:, 0:3, :], iuo[:, 3:6, :])
                fc4 = fc.rearrange("p c (n two) -> p c n two", two=2)
                t1 = scr.tile([128, 3, P], f32, tag="t1")
                nc.vector.tensor_add(t1, iu, fc4[:, :, :, 0])
                nc.vector.tensor_add(cc[:, :, pc : pc + P], t1, fc4[:, :, :, 1])

                # h = o * tanh(c)   (stored bf16 for the next level's matmuls)
                th = scr.tile([128, 3, P], f32, tag="th")
                nc.scalar.activation(out=th, in_=cc[:, :, pc : pc + P], func=AF.Tanh)
                nc.vector.tensor_mul(hb[:, :, pc : pc + P], iuo[:, 6:9, :], th)

            # ---- logits + log-softmax for all 127 local nodes ----
            # ones row at augmented position 300 (= chunk 2, partition 44) so
            # the W_out matmul adds b_out.
            nc.vector.memset(hb[64:65, 2, :], 1.0)
            for kc in range(3):
                kda = KD_AUG[kc]
                nc.tensor.matmul(
                    lg[:LOCAL_N, :],
                    hb[:kda, kc, :],
                    wo[:kda, kc, :],
                    start=(kc == 0),
                    stop=(kc == 2),
                )
            mx = scr.tile([128, 1], f32, tag="mx")
            nc.vector.reduce_max(mx[:LOCAL_N], lg[:LOCAL_N, :], axis=mybir.AxisListType.X)
            nmx = scr.tile([128, 1], f32, tag="nmx")
            nc.vector.tensor_scalar_mul(nmx[:LOCAL_N], mx[:LOCAL_N], -1.0)
            tshift = scr.tile([128, LBL], f32, tag="tshift")
            nc.vector.tensor_scalar_add(tshift[:LOCAL_N], lg[:LOCAL_N, :], nmx[:LOCAL_N])
            ex = scr.tile([128, LBL], f32, tag="ex")
            nc.scalar.activation(out=ex[:LOCAL_N], in_=tshift[:LOCAL_N], func=AF.Exp)
            sm = scr.tile([128, 1], f32, tag="sm")
            nc.vector.reduce_sum(sm[:LOCAL_N], ex[:LOCAL_N], axis=mybir.AxisListType.X)
            lse = scr.tile([128, 1], f32, tag="lse")
            nc.scalar.activation(out=lse[:LOCAL_N], in_=sm[:LOCAL_N], func=AF.Ln)
            lp = scr.tile([128, LBL], f32, tag="lp")
            nc.vector.tensor_scalar_sub(lp[:LOCAL_N], tshift[:LOCAL_N], lse[:LOCAL_N])
            nc.sync.dma_start(out=logp_d[:, :], in_=lp[:LOCAL_N, :])

            # ---- subtree root h, c back to host (fp32) ----
            rh = scr.tile([128, 3], f32, tag="rh")
            nc.vector.tensor_copy(rh, hb[:, :, LOCAL_N - 1])
            nc.sync.dma_start(out=rooth_d[:, :], in_=rh[:, :])
            nc.sync.dma_start(out=rootc_d[:, :], in_=cc[:, :, LOCAL_N - 1])

    nc.compile()
    return nc


def _get_compiled():
    if "nc" not in _compiled:
        _compiled["nc"] = _build_bass()
    return _compiled["nc"]


def _core_nodes(k):
    """Global node indices of core k's subtree, in local (level) order."""
    idx = []
    start, size = 0, 512
    for lvl in range(7):
        per = size // NCORES
        idx.append(np.arange(start + per * k, start + per * (k + 1)))
        start += size
        size //= 2
    return np.concatenate(idx)


def _sigmoid(z):
    return 1.0 / (1.0 + np.exp(-z))


def kernel(
    word_ids,
    labels,
    children_idx,
    children_mask,
    emb,
    W_ix,
    b_ix,
    W_ih,
    b_ih,
    W_fx,
    b_fx,
    W_fh,
    b_fh,
    W_ux,
    b_ux,
    W_uh,
    b_uh,
    W_out,
    b_out,
):
    global LAST_RESULTS
    from concourse import bass_utils

    word_ids = np.asarray(word_ids)
    labels = np.asarray(labels)
    children_idx = np.asarray(children_idx)
    children_mask = np.asarray(children_mask)
    emb = np.asarray(emb, dtype=np.float32)
    W = {
        "ix": np.asarray(W_ix, np.float32),
        "ih": np.asarray(W_ih, np.float32),
        "fx": np.asarray(W_fx, np.float32),
        "fh": np.asarray(W_fh, np.float32),
        "ux": np.asarray(W_ux, np.float32),
        "uh": np.asarray(W_uh, np.float32),
        "out": np.asarray(W_out, np.float32),
    }
    b = {
        "ix": np.asarray(b_ix, np.float32),
        "ih": np.asarray(b_ih, np.float32),
        "fx": np.asarray(b_fx, np.float32),
        "fh": np.asarray(b_fh, np.float32),
        "ux": np.asarray(b_ux, np.float32),
        "uh": np.asarray(b_uh, np.float32),
        "out": np.asarray(b_out, np.float32),
    }

    x = emb[word_ids]  # [1023, 300] host embedding gather

    # shared (replicated) weight uploads, bias folded in as a ones-row
    bf = ml_dtypes.bfloat16
    wa_np = {}
    for g, wx, wh_, bx, bh in (
        ("i", "ix", "ih", "ix", "ih"),
        ("u", "ux", "uh", "ux", "uh"),
        ("f", "fx", "fh", "fx", "fh"),
    ):
        wa_np[g] = np.vstack(
            [W[wx], np.zeros((20, H), np.float32), (b[bx] + b[bh])[None, :]]
        ).astype(bf)
    wh_np = {g: W[g + "h"].astype(bf) for g in "iuf"}
    wo_np = np.vstack(
        [W["out"], np.zeros((20, LBL), np.float32), b["out"][None, :]]
    ).astype(bf)

    node_lists = [_core_nodes(k) for k in range(NCORES)]
    in_maps = []
    for k in range(NCORES):
        xa = np.zeros((AUG, LOCAL_N), np.float32)
        xa[:300] = x[node_lists[k]].T
        xa[320] = 1.0
        m = {"xt": xa.astype(bf), "wo": wo_np}
        for g in "iuf":
            m[f"wa_{g}"] = wa_np[g]
            m[f"wh_{g}"] = wh_np[g]
        in_maps.append(m)

    nc = _get_compiled()
    res = bass_utils.run_bass_kernel_spmd(nc, in_maps, core_ids=list(range(NCORES)))
    LAST_RESULTS = res

    logp = np.empty((N, LBL), np.float32)
    h_top = {}
    c_top = {}
    for k in range(NCORES):
        out = res.results[k]
        logp[node_lists[k]] = out["logp"]
        h_top[1008 + k] = out["rooth"].T.reshape(-1)[:300].astype(np.float32)
        c_top[1008 + k] = out["rootc"].T.reshape(-1)[:300].astype(np.float32)

    # ---- top 7 nodes on host, fp32, faithful to the reference math ----
    for t in range(1016, 1023):
        ch = children_idx[t]
        hl, hr = h_top[ch[0]], h_top[ch[1]]
        cl, cr = c_top[ch[0]], c_top[ch[1]]
        hsum = hl + hr
        xt_row = x[t]
        ixx = xt_row @ W["ix"] + b["ix"]
        fxx = xt_row @ W["fx"] + b["fx"]
        uxx = xt_row @ W["ux"] + b["ux"]
        i = _sigmoid(ixx + hsum @ W["ih"] + b["ih"])
        o = _sigmoid(fxx + hsum @ W["fh"] + b["fh"])
        u = _sigmoid(uxx + hsum @ W["uh"] + b["uh"])
        fl = _sigmoid(hl @ W["fh"] + b["fh"] + fxx)
        fr = _sigmoid(hr @ W["fh"] + b["fh"] + fxx)
        c = i * u + fl * cl + fr * cr
        h = o * np.tanh(c)
        h_top[t] = h.astype(np.float32)
        c_top[t] = c.astype(np.float32)
        lgts = h @ W["out"] + b["out"]
        m = lgts.max()
        lsumexp = m + np.log(np.exp(lgts - m).sum())
        logp[t] = (lgts - lsumexp).astype(np.float32)

    loss = np.float32(-(logp[np.arange(N), labels].astype(np.float64).sum()))
    return logp, loss


# revision 7
# speedup vs baseline: 1.4796x; 1.4796x over previous
"""ChildSum TreeLSTM (complete binary tree, 1023 nodes) on 8 trn2 NeuronCores.

Strategy: the 1023-node complete binary tree splits into 8 independent
127-node subtrees rooted at the 8 nodes of the level with 8 nodes; each
NeuronCore runs one subtree (data-parallel, zero cross-core traffic).  The
top 7 nodes are finished on the host in fp32.  On device, activations live
in transposed layout [H=300 -> 3 partition-chunks of 128, nodes] so the
level recurrence needs no transposes and child pair-sums are stride-2
free-dim adds.  Gate pre-activations (x-projections + biases) are computed
once into a persistent PSUM tensor by upfront matmuls (bias folded in via a
ones-row of the padded input), and the per-level hidden-state matmuls
accumulate into the same PSUM region, so i/u need a single fused sigmoid.
All device tensors are zero-padded to H=384 so every matmul chunk is a full
128x128 (enables the PE fast-weight-load path and leaves no uninitialized
PSUM).  Matmul operands are bf16; cell state and elementwise math are fp32.
Logits come back raw; log-softmax and the NLL loss are host-side.
"""

import os
import sys

for _p in ("/opt/trn_rl_repo",):
    if _p not in sys.path and os.path.isdir(_p):
        sys.path.append(_p)

import numpy as np
import ml_dtypes

V = 100000
E = 300
H = 300
LBL = 5
N = 1023
NCORES = 8
LPC = 64  # leaves per core
LOCAL_N = 127  # nodes per core subtree
SIZES = [64, 32, 16, 8, 4, 2, 1]  # per-core level sizes
BASES = [0, 64, 96, 112, 120, 124, 126]  # local level start offsets

PAD = 384  # H padded to 3 full 128-chunks
ONES_ROW = 320  # bias/ones row position inside the zero padding (chunk 2, part 64)

_compiled = {}
LAST_RESULTS = None  # BassKernelResults of the most recent device run


def _build_bass():
    import concourse.bacc as bacc
    import concourse.mybir as mybir
    import concourse.tile as tile

    f32 = mybir.dt.float32
    bf16 = mybir.dt.bfloat16
    AF = mybir.ActivationFunctionType

    nc = bacc.Bacc()

    # DRAM I/O (per-core shapes; SPMD with per-core in_maps)
    xt_d = nc.dram_tensor("xt", [PAD, LOCAL_N], bf16, kind="ExternalInput")
    wa_d = {
        g: nc.dram_tensor(f"wa_{g}", [PAD, PAD], bf16, kind="ExternalInput")
        for g in ("i", "u", "f")
    }
    wh_d = {
        g: nc.dram_tensor(f"wh_{g}", [PAD, PAD], bf16, kind="ExternalInput")
        for g in ("i", "u", "f")
    }
    wo_d = nc.dram_tensor("wo", [PAD, LBL], bf16, kind="ExternalInput")
    logits_d = nc.dram_tensor("logits", [LOCAL_N, LBL], f32, kind="ExternalOutput")
    rooth_d = nc.dram_tensor("rooth", [128, 3], f32, kind="ExternalOutput")
    rootc_d = nc.dram_tensor("rootc", [128, 3], f32, kind="ExternalOutput")

    with tile.TileContext(nc) as tc:
        with (
            tc.tile_pool(name="const", bufs=1) as const,
            tc.tile_pool(name="state", bufs=1) as state,
            tc.tile_pool(name="scr", bufs=2) as scr,
            tc.tile_pool(name="pprex", bufs=1, space="PSUM") as pprex,
            tc.tile_pool(name="plg", bufs=1, space="PSUM") as plg,
        ):
            # ---- constants in SBUF (one consolidated DMA per tensor) ----
            xt = const.tile([128, 3, LOCAL_N], bf16, tag="xt")
            wa = {g: const.tile([128, 3, PAD], bf16, tag=f"wa{g}", name=f"wa{g}") for g in "iuf"}
            wh = {g: const.tile([128, 3, PAD], bf16, tag=f"wh{g}", name=f"wh{g}") for g in "iuf"}
            wo = const.tile([128, 3, LBL], bf16, tag="wo")

            nc.sync.dma_start(
                out=xt[:, :, :], in_=xt_d.rearrange("(c p) n -> p c n", p=128)
            )
            for g in "iuf":
                nc.sync.dma_start(
                    out=wa[g][:, :, :],
                    in_=wa_d[g].rearrange("(c p) m -> p c m", p=128),
                )
            for g in "iuf":
                nc.sync.dma_start(
                    out=wh[g][:, :, :],
                    in_=wh_d[g].rearrange("(c p) m -> p c m", p=128),
                )
            nc.sync.dma_start(
                out=wo[:, :, :], in_=wo_d.rearrange("(c p) m -> p c m", p=128)
            )

            # ---- persistent state ----
            hb = state.tile([128, 3, LOCAL_N], bf16, tag="hb")  # hidden (bf16)
            cc = state.tile([128, 3, LOCAL_N], f32, tag="cc")  # cell (fp32)
            prexf = state.tile([128, 3, LOCAL_N], f32, tag="prexf")  # fxx+b copy

            # ---- PSUM ----
            # gate pre-activations, lanes (gate*3 + mchunk); gates i,u,f
            prex = pprex.tile([128, 9, 128], f32, tag="prex")
            # per-child forget pre-activation scratch
            gp = pprex.tile([128, 3, 2 * SIZES[1]], f32, tag="gp")
            lg = plg.tile([128, LBL], f32, tag="lg")

            # upfront x-projections (+bias via ones row) for all 127 nodes
            for gi, g in enumerate("iuf"):
                for mc in range(3):
                    for kc in range(3):
                        nc.tensor.matmul(
                            prex[:, gi * 3 + mc, :LOCAL_N],
                            wa[g][:, kc, mc * 128 : (mc + 1) * 128],
                            xt[:, kc, :],
                            start=(kc == 0),
                            stop=(kc == 2),
                        )

            # SBUF copy of the f-gate x-projection (fxx + b_fx + b_fh): the
            # per-level f/o adds need it as the non-PSUM operand.
            nc.vector.tensor_copy(prexf[:, :, :], prex[:, 6:9, :LOCAL_N])

            # ---- leaves: no children; i,u,o are pure x-projections ----
            iuo_l = scr.tile([128, 9, LPC], f32, tag="iuo")
            nc.scalar.activation(out=iuo_l, in_=prex[:, :, :LPC], func=AF.Sigmoid)
            nc.vector.tensor_mul(cc[:, :, :LPC], iuo_l[:, 0:3, :], iuo_l[:, 3:6, :])
            th_l = scr.tile([128, 3, LPC], f32, tag="th")
            nc.scalar.activation(out=th_l, in_=cc[:, :, :LPC], func=AF.Tanh)
            nc.vector.tensor_mul(hb[:, :, :LPC], iuo_l[:, 6:9, :], th_l)

            # ---- internal levels ----
            for lvl in range(1, 7):
                P = SIZES[lvl]
                C = 2 * P
                cb = BASES[lvl - 1]
                pc = BASES[lvl]

                # child-sum of h (bf16, matmul rhs)
                hsum = scr.tile([128, 3, P], bf16, tag="hsum")
                chv = hb[:, :, cb : cb + C].rearrange(
                    "p c (n two) -> p c n two", two=2
                )
                nc.vector.tensor_add(hsum, chv[:, :, :, 0], chv[:, :, :, 1])

                # i, u recurrence matmuls accumulate onto the PSUM
                # pre-activations
                for gi, g in enumerate("iu"):
                    for mc in range(3):
                        for kc in range(3):
                            nc.tensor.matmul(
                                prex[:, gi * 3 + mc, pc : pc + P],
                                wh[g][:, kc, mc * 128 : (mc + 1) * 128],
                                hsum[:, kc, :],
                                start=False,
                                stop=(kc == 2),
                                skip_group_check=True,
                            )

                # per-child forget pre-activation g = h_child @ W_fh
                for mc in range(3):
                    for kc in range(3):
                        nc.tensor.matmul(
                            gp[:, mc, :C],
                            wh["f"][:, kc, mc * 128 : (mc + 1) * 128],
                            hb[:, kc, cb : cb + C],
                            start=(kc == 0),
                            stop=(kc == 2),
                        )

                # fused sigmoid over i/u lanes for this level's columns
                iu = scr.tile([128, 6, P], f32, tag="iuo")
                nc.scalar.activation(
                    out=iu, in_=prex[:, 0:6, pc : pc + P], func=AF.Sigmoid
                )

                # f (per child) and o (per parent) pre-activations packed in
                # one tile -> one sigmoid:
                #   fo[:, :, :C]  = g + fxx_parent (broadcast to the pair)
                #   fo[:, :, C:]  = tmpf_even + g_odd  (= g_e + g_o + fxx)
                fo = scr.tile([128, 3, C + P], f32, tag="fo")
                gp4 = gp[:, :, :C].rearrange("p c (n two) -> p c n two", two=2)
                fo4 = fo[:, :, :C].rearrange("p c (n two) -> p c n two", two=2)
                nc.vector.tensor_add(
                    fo4, gp4, prexf[:, :, pc : pc + P].to_broadcast((128, 3, P, 2))
                )
                nc.vector.tensor_add(
                    fo[:, :, C : C + P], fo[:, :, 0:C:2], gp[:, :, 1:C:2]
                )
                fos = scr.tile([128, 3, C + P], f32, tag="fos")
                nc.scalar.activation(out=fos, in_=fo, func=AF.Sigmoid)

                # c = i*u + sum_children f*c_child
                fc = scr.tile([128, 3, C], f32, tag="fc")
                nc.vector.tensor_mul(fc, fos[:, :, :C], cc[:, :, cb : cb + C])
                iup = scr.tile([128, 3, P], f32, tag="iup")
                nc.vector.tensor_mul(iup, iu[:, 0:3, :], iu[:, 3:6, :])
                fc4 = fc.rearrange("p c (n two) -> p c n two", two=2)
                t1 = scr.tile([128, 3, P], f32, tag="t1")
                nc.vector.tensor_add(t1, iup, fc4[:, :, :, 0])
                nc.vector.tensor_add(cc[:, :, pc : pc + P], t1, fc4[:, :, :, 1])

                # h = o * tanh(c)   (stored bf16 for the next level's matmuls)
                th = scr.tile([128, 3, P], f32, tag="th")
                nc.scalar.activation(out=th, in_=cc[:, :, pc : pc + P], func=AF.Tanh)
                nc.vector.tensor_mul(hb[:, :, pc : pc + P], fos[:, :, C : C + P], th)

            # ---- logits for all 127 local nodes (softmax is host-side) ----
            # ones row at padded position 320 (= chunk 2, partition 64) so the
            # W_out matmul adds b_out.
            nc.vector.memset(hb[64:65, 2, :], 1.0)
            for kc in range(3):
                nc.tensor.matmul(
                    lg[:LOCAL_N, :],
                    hb[:, kc, :],
                    wo[:, kc, :],
                    start=(kc == 0),
                    stop=(kc == 2),
                )
            lgs = scr.tile([128, LBL], f32, tag="lgs")
            nc.vector.tensor_copy(lgs[:LOCAL_N, :], lg[:LOCAL_N, :])
            nc.sync.dma_start(out=logits_d[:, :], in_=lgs[:LOCAL_N, :])

            # ---- subtree root h, c back to host (fp32) ----
            rh = scr.tile([128, 3], f32, tag="rh")
            nc.vector.tensor_copy(rh, hb[:, :, LOCAL_N - 1])
            nc.sync.dma_start(out=rooth_d[:, :], in_=rh[:, :])
            nc.sync.dma_start(out=rootc_d[:, :], in_=cc[:, :, LOCAL_N - 1])

    nc.compile()
    return nc


def _get_compiled():
    if "nc" not in _compiled:
        _compiled["nc"] = _build_bass()
    return _compiled["nc"]


def _core_nodes(k):
    """Global node indices of core k's subtree, in local (level) order."""
    idx = []
    start, size = 0, 512
    for lvl in range(7):
        per = size // NCORES
        idx.append(np.arange(start + per * k, start + per * (k + 1)))
        start += size
        size //= 2
    return np.concatenate(idx)


def _sigmoid(z):
    return 1.0 / (1.0 + np.exp(-z))


def kernel(
    word_ids,
    labels,
    children_idx,
    children_mask,
    emb,
    W_ix,
    b_ix,
    W_ih,
    b_ih,
    W_fx,
    b_fx,
    W_fh,
    b_fh,
    W_ux,
    b_ux,
    W_uh,
    b_uh,
    W_out,
    b_out,
):
    global LAST_RESULTS
    from concourse import bass_utils

    word_ids = np.asarray(word_ids)
    labels = np.asarray(labels)
    children_idx = np.asarray(children_idx)
    children_mask = np.asarray(children_mask)
    emb = np.asarray(emb, dtype=np.float32)
    W = {
        "ix": np.asarray(W_ix, np.float32),
        "ih": np.asarray(W_ih, np.float32),
        "fx": np.asarray(W_fx, np.float32),
        "fh": np.asarray(W_fh, np.float32),
        "ux": np.asarray(W_ux, np.float32),
        "uh": np.asarray(W_uh, np.float32),
        "out": np.asarray(W_out, np.float32),
    }
    b = {
        "ix": np.asarray(b_ix, np.float32),
        "ih": np.asarray(b_ih, np.float32),
        "fx": np.asarray(b_fx, np.float32),
        "fh": np.asarray(b_fh, np.float32),
        "ux": np.asarray(b_ux, np.float32),
        "uh": np.asarray(b_uh, np.float32),
        "out": np.asarray(b_out, np.float32),
    }

    x = emb[word_ids]  # [1023, 300] host embedding gather

    # shared (replicated) weight uploads, zero-padded to 384 with the
    # combined bias folded in at the ones-row position
    bf = ml_dtypes.bfloat16
    wa_np = {}
    for g, wx, whk in (("i", "ix", "ih"), ("u", "ux", "uh"), ("f", "fx", "fh")):
        m = np.zeros((PAD, PAD), np.float32)
        m[:H, :H] = W[wx]
        m[ONES_ROW, :H] = b[wx] + b[whk]
        wa_np[g] = m.astype(bf)
    wh_np = {}
    for g in "iuf":
        m = np.zeros((PAD, PAD), np.float32)
        m[:H, :H] = W[g + "h"]
        wh_np[g] = m.astype(bf)
    m = np.zeros((PAD, LBL), np.float32)
    m[:H] = W["out"]
    m[ONES_ROW] = b["out"]
    wo_np = m.astype(bf)

    node_lists = [_core_nodes(k) for k in range(NCORES)]
    in_maps = []
    for k in range(NCORES):
        xa = np.zeros((PAD, LOCAL_N), np.float32)
        xa[:300] = x[node_lists[k]].T
        xa[ONES_ROW] = 1.0
        m = {"xt": xa.astype(bf), "wo": wo_np}
        for g in "iuf":
            m[f"wa_{g}"] = wa_np[g]
            m[f"wh_{g}"] = wh_np[g]
        in_maps.append(m)

    nc = _get_compiled()
    res = bass_utils.run_bass_kernel_spmd(nc, in_maps, core_ids=list(range(NCORES)))
    LAST_RESULTS = res

    logits = np.empty((N, LBL), np.float32)
    h_top = {}
    c_top = {}
    for k in range(NCORES):
        out = res.results[k]
        logits[node_lists[k]] = out["logits"]
        h_top[1008 + k] = out["rooth"].T.reshape(-1)[:300].astype(np.float32)
        c_top[1008 + k] = out["rootc"].T.reshape(-1)[:300].astype(np.float32)

    # ---- top 7 nodes on host, fp32, faithful to the reference math ----
    for t in range(1016, 1023):
        ch = children_idx[t]
        hl, hr = h_top[ch[0]], h_top[ch[1]]
        cl, cr = c_top[ch[0]], c_top[ch[1]]
        hsum = hl + hr
        xt_row = x[t]
        ixx = xt_row @ W["ix"] + b["ix"]
        fxx = xt_row @ W["fx"] + b["fx"]
        uxx = xt_row @ W["ux"] + b["ux"]
        i = _sigmoid(ixx + hsum @ W["ih"] + b["ih"])
        o = _sigmoid(fxx + hsum @ W["fh"] + b["fh"])
        u = _sigmoid(uxx + hsum @ W["uh"] + b["uh"])
        fl = _sigmoid(hl @ W["fh"] + b["fh"] + fxx)
        fr = _sigmoid(hr @ W["fh"] + b["fh"] + fxx)
        c = i * u + fl * cl + fr * cr
        h = o * np.tanh(c)
        h_top[t] = h.astype(np.float32)
        c_top[t] = c.astype(np.float32)
        logits[t] = (h @ W["out"] + b["out"]).astype(np.float32)

    # log-softmax + NLL on host (exact, vectorized)
    mx = logits.max(axis=1, keepdims=True)
    ex = np.exp(logits - mx)
    logp = (logits - mx) - np.log(ex.sum(axis=1, keepdims=True))
    logp = logp.astype(np.float32)
    loss = np.float32(-(logp[np.arange(N), labels].astype(np.float64).sum()))
    return logp, loss
